# revision 1
# baseline (speedup 1.0000x reference)
"""Trainium2 Bass kernel for ComplexCoherency loss.

Reference computation (per full input [B=16, C=2, H=512, W=512], k=5):
    num_r = box5x5(sum_c(pr*tr + pi*ti))     [B,508,508]
    num_i = box5x5(sum_c(pi*tr - pr*ti))
    den_p = box5x5(sum_c(pr^2 + pi^2))
    den_t = box5x5(sum_c(tr^2 + ti^2))
    abs_c = sqrt(num_r^2+num_i^2) / sqrt(den_p*den_t)
    out   = 1 - mean(abs_c)

Sharding: pure data parallel, B=16 split 2-per-core across 8 NeuronCores.
Each core returns per-partition partial sums of abs_c; host reduces.

Engine split (elementwise ops cost free-size cycles; the partition dim is
free, so every map is laid out as [128 rows, blocks x 512 cols]):
  PE  : 5-row H-box + channel/term sum fused as banded bf16 matmuls
        accumulating into PSUM (4 main + 4 next-block-boundary bands per
        128-row block)
  DVE : most of each product mul (column-split with Pool), W-box cumsum
        scans (PSUM-reading ops are DVE/ACT-only on this ISA) and shifted
        subtracts (bf16 2x mode for num maps; fp32 for den maps whose
        monotone cumsum would cancel catastrophically in bf16), bf16
        finals muls
  Pool: the tail columns of every product mul (gpsimd tensor_mul)
  ACT : input squares, Ln/Exp finals exp(0.5(ln r - ln s)) with accum_out
        summing abs_c.  A pre-emitted LoadActFuncSet pins the one act
        table holding Square+Ln+Exp so the compiler inserts no reloads.

The 8 (batch, map) stages are software-pipelined: stage k's matmuls are
emitted with stage k+2's products and stage k-1's W-box, and batch-0
finals are deferred past the mid-stream DVE congestion.  PSUM rotates as
half-image tiles (2 banks, bufs=4); post-H-box chains are half-split so
the final stage drains per-half.  DMA order follows consumption (pred
tensors before tgt, batch-0 leading transfers halved) to start PE by
~6 us and keep the input stream gapless.
"""

import numpy as np
import ml_dtypes
from contextlib import ExitStack

import concourse.bass as bass
import concourse.bacc as bacc
import concourse.tile as tile
import concourse.mybir as mybir
from concourse.bass_utils import run_bass_kernel_spmd
from concourse.hw_specs import get_activation_tables

B, C, H, W = 16, 2, 512, 512
KF = 5                      # filter size (hardcoded)
NCORES = 8
BLOC = B // NCORES          # 2 batches per core
HP = H - KF + 1             # 508
WP = W - KF + 1             # 508
NBLK = H // 128             # 4 partition blocks per image
FD = NBLK * W               # 2048 free-dim elems per image-channel tile
HFD = FD // 2               # 1024 elems per half (2 blocks)
GW = NBLK * WP              # 2032 free-dim elems per box-filtered tile
HGW = GW // 2               # 1016 per half
TAIL_ROWS = HP - 3 * 128    # 124 valid rows in block 3

AF = mybir.ActivationFunctionType
ALU = mybir.AluOpType
F32 = mybir.dt.float32
BF16 = mybir.dt.bfloat16

# Column split of each product mul: [0:c] on DVE, [c:FD] on Pool, per batch.
MULC = {0: 1088, 1: 832}

MAPS = ["den_p", "num_r", "num_i", "den_t"]
NUMS = ("num_r", "num_i")
# product spec per map: (a, b) factor names per channel
PSPEC = {"num_r": (("pr", "tr"), ("pi", "ti")),
         "num_i": (("pi", "tr"), ("pr", "ti")),
         "den_p": (("pr", "pr"), ("pi", "pi")),
         "den_t": (("tr", "tr"), ("ti", "ti"))}
SIGNS = {"num_r": (0, 0), "num_i": (0, 1), "den_p": (0, 0), "den_t": (0, 0)}


def _make_bands() -> np.ndarray:
    """[4,128,128] bf16: +main, +boundary, -main, -boundary H-box bands."""
    k = np.arange(128)[:, None]
    m = np.arange(128)[None, :]
    main = ((k - m >= 0) & (k - m <= KF - 1)).astype(np.float32)
    bnd = np.zeros((128, 128), np.float32)
    for kk in range(KF - 1):
        bnd[kk, 124 + kk:] = 1.0
    return np.stack([main, bnd, -main, -bnd]).astype(ml_dtypes.bfloat16)


def _act_set_id(nc) -> int:
    """Index of the first act-func table containing Square, Ln and Exp."""
    want = {AF.Square, AF.Ln, AF.Exp}
    for i, (name, s) in enumerate(get_activation_tables(nc.m.arch).items()):
        if want <= s:
            return i
    raise RuntimeError("no act table with Square+Ln+Exp")


def _build_nc() -> bacc.Bacc:
    nc = bacc.Bacc("TRN2", target_bir_lowering=False, debug=False,
                   num_devices=NCORES)
    ins = {
        name: nc.dram_tensor(name, [BLOC, C, H, W], F32,
                             kind="ExternalInput").ap()
        for name in ("pred_real", "pred_imag", "tgt_real", "tgt_imag")
    }
    bands = nc.dram_tensor("bands", [4, 128, 128], BF16,
                           kind="ExternalInput").ap()
    out = nc.dram_tensor("partials", [128, 3 * BLOC], F32,
                         kind="ExternalOutput").ap()
    with tile.TileContext(nc) as tc, ExitStack() as ctx:
        _kernel(ctx, tc, out, ins, bands)
    nc.compile()
    return nc


def _kernel(ctx, tc, out_ap, ins, bands_dram):
    nc = tc.nc
    pool = ctx.enter_context(tc.tile_pool(name="main", bufs=2))
    psum_pool = ctx.enter_context(tc.tile_pool(name="psum", space="PSUM",
                                               bufs=4))

    # ---- constants / persistent state -----------------------------------
    nc.scalar.add_instruction(mybir.InstLoadActFuncSet(
        name=nc.get_next_instruction_name(), ins=[], outs=[],
        act_func_set_id=_act_set_id(nc)))

    bands_sb = pool.tile([128, 4, 128], BF16, tag="bands", bufs=1)
    nc.sync.dma_start(out=bands_sb,
                      in_=bands_dram.rearrange("i k m -> k i m"))
    band_main = (bands_sb[:, 0, :], bands_sb[:, 2, :])    # (+, -) [128,128]
    band_bnd = (bands_sb[0:4, 1, :], bands_sb[0:4, 3, :])  # (+, -) [4,128]

    zeros = pool.tile([128, HFD], F32, tag="zeros", bufs=1)
    nc.gpsimd.memset(zeros, 0.0)
    acc = pool.tile([128, 3 * BLOC], F32, tag="acc", bufs=1)
    nc.gpsimd.memset(acc, 0.0)

    # Persistent cumsum tiles, one per (numeric kind, half). The scan only
    # writes cols 1:, so the zero in col 0 (memset once) persists across
    # stages; tile deps serialize scan(stage k) after sub(stage k-2).
    csn = [pool.tile([128, HFD + 1], BF16, tag=f"csn{h}", bufs=1,
                     name=f"csn{h}") for h in range(2)]
    for cs in csn:
        nc.gpsimd.memset(cs[:, 0:1], 0.0)

    img = {}    # (b, nm, c) -> input tile [128, 4, 512] fp32
    prods = {}  # (b, mname) -> [(tile, sign), ...] in emission order
    GB = {b: {} for b in range(BLOC)}
    RB = {}

    def products(b, mname):
        """Emit product tiles for one stage (both channels; bf16 out)."""
        for c in range(C):
            for (a, bb), sg in zip(PSPEC[mname], SIGNS[mname]):
                pt = pool.tile([128, NBLK, W], BF16, tag="prod", bufs=10)
                if a == bb:
                    if b == 0 and c == 0 and mname == "den_p":
                        # halved to chase the halved first DMAs
                        for hh in range(2):
                            ss = slice(2 * hh, 2 * hh + 2)
                            nc.scalar.activation(out=pt[:, ss, :],
                                                 in_=img[(b, a, c)][:, ss, :],
                                                 func=AF.Square)
                    else:
                        nc.scalar.activation(out=pt, in_=img[(b, a, c)],
                                             func=AF.Square)
                else:
                    x = img[(b, a, c)].rearrange("p t w -> p (t w)")
                    y = img[(b, bb, c)].rearrange("p t w -> p (t w)")
                    d = pt.rearrange("p t w -> p (t w)")
                    mc = MULC[b]
                    nc.vector.tensor_mul(d[:, 0:mc], x[:, 0:mc],
                                         y[:, 0:mc])
                    nc.gpsimd.tensor_mul(d[:, mc:FD], x[:, mc:FD],
                                         y[:, mc:FD])
                prods.setdefault((b, mname), []).append((pt, sg))

    def hbox(b, mname):
        """H-box matmuls -> two half psum tiles [128, 2, 512] (2 banks).

        Channel-0 mains first so PE fills while channel 1 still loads;
        boundary matmuls last carry the accumulation-group stop flags.
        """
        ptiles = prods.pop((b, mname))   # 4 tiles: c0 pair, c1 pair
        ph = [psum_pool.tile([128, 2, 512], F32, tag="ps", bufs=4,
                             name=f"ps_{b}_{mname}_{h}") for h in range(2)]
        started = set()
        mm = []
        for t in range(NBLK):                      # block-major: half 0
            for pt, sg in ptiles:                  # completes first so its
                mm.append((t, band_main[sg], pt[:, t, :]))   # scan can start
        for t in range(NBLK - 1):
            for pt, sg in ptiles:
                mm.append((t, band_bnd[sg], pt[0:4, t + 1, :]))
        last_idx = {}
        for i, (t, _, _) in enumerate(mm):
            last_idx[t] = i
        for i, (t, lhsT, rhs) in enumerate(mm):
            outband = ph[t // 2][:, t % 2, :]
            nc.tensor.matmul(outband, lhsT, rhs,
                             start=(t not in started),
                             stop=(last_idx[t] == i))
            started.add(t)
        return ph

    def wbox(b, mname, ph):
        """W-box per half.

        num maps: DVE cumsum scan (bf16 out) + bf16 2x-mode shifted sub.
        den maps: ACT psum->bf16 copy + 3 shifted bf16 adds.  The den
        cumsum is monotone (bf16 differences would cancel), but the box
        values themselves are positive and bf16-safe; this keeps the
        fp32 scan+sub off the critical DVE stream.
        """
        num = mname in NUMS
        g = pool.tile([128, NBLK, WP], BF16, tag="g", bufs=7)
        for h in range(2):
            if num:
                cs = csn[h]
                with nc.allow_low_precision(reason="bf16 cumsum; box diffs "
                                            "average out in the loss mean"):
                    nc.vector.tensor_tensor_scan(
                        out=cs[:, 1:HFD + 1],
                        data0=ph[h].rearrange("p t w -> p (t w)"),
                        data1=zeros, initial=0.0, op0=ALU.add, op1=ALU.add)
                c3 = cs[:, 1:].rearrange("p (t w) -> p t w", t=2)
                c0 = cs[:, 0:HFD].rearrange("p (t w) -> p t w", t=2)
                nc.vector.tensor_sub(
                    g[:, 2 * h:2 * h + 2, :], c3[:, :, KF - 1:W],
                    c0[:, :, 0:WP])
            else:
                y = pool.tile([128, HFD], BF16, tag="y", bufs=2)
                nc.scalar.activation(
                    out=y, in_=ph[h].rearrange("p t w -> p (t w)"),
                    func=AF.Copy)
                s2 = pool.tile([128, HFD], BF16, tag="s2", bufs=2)
                nc.vector.tensor_add(s2[:, 0:HFD - 1], y[:, 0:HFD - 1],
                                     y[:, 1:HFD])
                s4 = pool.tile([128, HFD], BF16, tag="s4", bufs=2)
                nc.vector.tensor_add(s4[:, 0:HFD - 3], s2[:, 0:HFD - 3],
                                     s2[:, 2:HFD - 1])
                y3 = y.rearrange("p (t w) -> p t w", t=2)
                s43 = s4.rearrange("p (t w) -> p t w", t=2)
                nc.vector.tensor_add(
                    g[:, 2 * h:2 * h + 2, :], s43[:, :, 0:WP],
                    y3[:, :, KF - 1:W])
        GB[b][mname] = g

    def fmul(dst, x, y, add=False):
        for h in range(2):
            sl = slice(2 * h, 2 * h + 2)
            op = nc.vector.tensor_add if add else nc.vector.tensor_mul
            op(dst[:, sl, :], x[:, sl, :], y[:, sl, :])

    def post_wbox(b, mname):
        """Finals pieces as soon as their inputs exist."""
        if mname == "num_i":
            t1 = pool.tile([128, NBLK, WP], BF16, tag="f", bufs=4)
            fmul(t1, GB[b]["num_r"], GB[b]["num_r"])
            t2 = pool.tile([128, NBLK, WP], BF16, tag="f", bufs=4)
            fmul(t2, GB[b]["num_i"], GB[b]["num_i"])
            r = pool.tile([128, NBLK, WP], BF16, tag="r", bufs=2)
            fmul(r, t1, t2, add=True)
            ln_r = pool.tile([128, NBLK, WP], BF16, tag="lnr", bufs=2)
            for h in range(2):
                sl = slice(2 * h, 2 * h + 2)
                nc.scalar.activation(out=ln_r[:, sl, :], in_=r[:, sl, :],
                                     func=AF.Ln)
            RB[b] = ln_r
        elif mname == "den_t":
            s = pool.tile([128, NBLK, WP], BF16, tag="f", bufs=4)
            fmul(s, GB[b]["den_p"], GB[b]["den_t"])
            ln_r = RB[b]
            d = pool.tile([128, NBLK, WP], BF16, tag="f", bufs=4)
            for h in range(2):
                sl = slice(2 * h, 2 * h + 2)
                ln_s = pool.tile([128, 2, WP], BF16, tag="lns", bufs=2)
                nc.scalar.activation(out=ln_s, in_=s[:, sl, :], func=AF.Ln)
                nc.vector.tensor_sub(d[:, sl, :], ln_r[:, sl, :], ln_s)
            # exp pieces: blocks 0-1, block 2, block-3 tail (124 rows)
            sink = pool.tile([128, 2 * WP], BF16, tag="sink", bufs=1)
            df = d.rearrange("p t w -> p (t w)")
            nc.scalar.activation(out=sink, in_=df[:, 0:HGW], func=AF.Exp,
                                 scale=0.5, accum_out=acc[:, 3 * b:3 * b + 1])
            nc.scalar.activation(out=sink[:, 0:WP], in_=df[:, HGW:HGW + WP],
                                 func=AF.Exp, scale=0.5,
                                 accum_out=acc[:, 3 * b + 1:3 * b + 2])
            nc.scalar.activation(out=sink[0:TAIL_ROWS, 0:WP],
                                 in_=df[0:TAIL_ROWS, HGW + WP:GW],
                                 func=AF.Exp, scale=0.5,
                                 accum_out=acc[0:TAIL_ROWS,
                                               3 * b + 2:3 * b + 3])

    # ---- pipeline --------------------------------------------------------
    # DMA order tracks consumption: per batch, channel 0 then channel 1,
    # (pr, pi, tr, ti) within each channel.
    for b in range(BLOC):
        for nm, key, c in (("pr", "pred_real", 0), ("pi", "pred_imag", 0),
                           ("pr", "pred_real", 1), ("pi", "pred_imag", 1),
                           ("tr", "tgt_real", 0), ("ti", "tgt_imag", 0),
                           ("tr", "tgt_real", 1), ("ti", "tgt_imag", 1)):
            if True:
                t = pool.tile([128, NBLK, W], F32, tag="in", bufs=10)
                src_ap = ins[key][b, c].rearrange("(t p) w -> p t w", p=128)
                if b == 0 and c == 0:
                    # halved first transfers: the first squares (den_p) and
                    # first matmuls start ~3us earlier
                    nc.sync.dma_start(out=t[:, 0:2, :], in_=src_ap[:, 0:2, :])
                    nc.sync.dma_start(out=t[:, 2:4, :], in_=src_ap[:, 2:4, :])
                else:
                    nc.sync.dma_start(out=t, in_=src_ap)
                img[(b, nm, c)] = t

    stages = [(b, m) for b in range(BLOC) for m in MAPS]
    pend = []
    finq = []
    products(*stages[0])
    products(*stages[1])
    for k, (b, m) in enumerate(stages):
        ph = hbox(b, m)
        pend.append((b, m, ph))
        if k >= 1:
            wbox(*pend.pop(0))
            finq.append(stages[k - 1])
            # batch-0 finals wait until DVE's mid-stream congestion passes
            if k >= 6:
                while finq:
                    post_wbox(*finq.pop(0))
        if k + 2 < len(stages):
            products(*stages[k + 2])
    wbox(*pend.pop(0))
    finq.append(stages[-1])
    while finq:
        post_wbox(*finq.pop(0))

    nc.sync.dma_start(out=out_ap[:, 0:3], in_=acc[:, 0:3])
    nc.sync.dma_start(out=out_ap[:, 3:6], in_=acc[:, 3:6])


_NC_CACHE = None


def _get_nc():
    global _NC_CACHE
    if _NC_CACHE is None:
        _NC_CACHE = _build_nc()
    return _NC_CACHE


def _run(inputs: dict, trace: bool = False, **kw):
    nc = _get_nc()
    bands = _make_bands()
    full = {k: np.ascontiguousarray(np.asarray(inputs[k]), dtype=np.float32)
            for k in ("pred_real", "pred_imag", "tgt_real", "tgt_imag")}
    in_maps = []
    for i in range(NCORES):
        sl = slice(i * BLOC, (i + 1) * BLOC)
        m = {k: np.ascontiguousarray(v[sl]) for k, v in full.items()}
        m["bands"] = bands
        in_maps.append(m)
    res = run_bass_kernel_spmd(nc, in_maps, core_ids=list(range(NCORES)),
                               trace=trace, **kw)
    total = 0.0
    for r in res.results:
        total += r["partials"].astype(np.float64).sum()
    coh = total / (B * HP * WP)
    out = np.float32(1.0 - coh)
    return np.asarray(out, dtype=np.float32), res


def kernel(pred_real, pred_imag, tgt_real, tgt_imag, filter_size):
    assert int(filter_size) == KF, f"filter_size {filter_size} != {KF}"
    out, _ = _run(dict(pred_real=pred_real, pred_imag=pred_imag,
                       tgt_real=tgt_real, tgt_imag=tgt_imag))
    return out

